# revision 1
# baseline (speedup 1.0000x reference)
"""DeepSet encoder (phi MLP -> sum/max pool -> rho MLP) as a Trainium2 Bass kernel.

Sharding: data-parallel over the batch dim. 64 samples -> 8 cores x 8 samples.
Weights are replicated on every core; no cross-core communication.

On-chip layout is feature-major ("transposed"): activations live as
[feature_partition, set_free] tiles so that
  - matmul contraction (over features) is on the partition dim,
  - the bias is a per-partition scalar (free on ScalarE's activation op),
  - sum/max pooling over the set dim is a free-axis reduction
    (sum comes for free via activation's accum_out).
The host pre-transposes x to [B, D_IN, N] and casts inputs to bf16.

Self-contained: only relies on the system-installed concourse/bass stack.
"""

import sys

import numpy as np

for _p in ("/opt/trn_rl_repo",):
    if _p not in sys.path:
        sys.path.insert(0, _p)

import ml_dtypes  # noqa: E402

import concourse.bass as bass  # noqa: E402,F401
import concourse.mybir as mybir  # noqa: E402
import concourse.tile as tile  # noqa: E402
from concourse import bacc  # noqa: E402
from concourse.bass_utils import run_bass_kernel_spmd  # noqa: E402

# 16-bit compute dtype: fp16 runs the PE at the same 1 cycle/row as bf16 but
# carries 10 mantissa bits instead of 8. All intermediates here are O(100) max,
# far inside fp16 range, so fp16 is a free 4x accuracy win over bf16.
BF16 = mybir.dt.float16
FP32 = mybir.dt.float32
NP_BF16 = np.float16
# phi1 runs in fp8e4m3 with DoubleRow: 2 fp8 weights per PE cell contract 256
# rows per pass, halving phi1's matmul count. x ~ N(0,1) and W1 ~ U(+-0.044)
# sit far inside TRN e4m3's +-240 range; measured end-to-end error with
# fp8-phi1 + fp16-rest is ~0.003 (same as an all-bf16 kernel).
FP8 = mybir.dt.float8e4
NP_FP8 = ml_dtypes.float8_e4m3
DOUBLE_ROW = mybir.MatmulPerfMode.DoubleRow

B, N, D_IN, D_H = 64, 512, 512, 1024
N_CORES = 8
BL = B // N_CORES  # samples per core
P = 128
K1 = D_IN // P  # phi1 contraction tiles (4)
K2 = D_H // P  # phi2/rho2 contraction tiles & D_H output tiles (8)
KR1 = 2 * D_H // P  # rho1 contraction tiles (16)
KK1 = D_IN // 256  # phi1 DoubleRow chunks (2)

RELU = mybir.ActivationFunctionType.Relu
AX_X = mybir.AxisListType.X
OP_MAX = mybir.AluOpType.max


def build_program() -> bacc.Bacc:
    nc = bacc.Bacc("TRN2", target_bir_lowering=False, debug=False, num_devices=N_CORES)

    # all staged host-side into the exact SBUF tile layouts so every DMA is
    # contiguous per partition (large descriptor runs):
    #   xt[b, p, kk, j, n] = x[b, n, kk*256 + j*128 + p]   (fp8, DoubleRow pairs)
    #   w1[p, kk, j, h] = W1[kk*256 + j*128 + p, h]        (fp8)
    #   w*[p, ko, h] = W[ko*128+p, h]                      (fp16)
    xt_d = nc.dram_tensor("xt", [BL, P, KK1, 2, N], FP8, kind="ExternalInput").ap()
    w1_d = nc.dram_tensor("w1", [P, KK1, 2, D_H], FP8, kind="ExternalInput").ap()
    w2_d = nc.dram_tensor("w2", [P, K2, D_H], BF16, kind="ExternalInput").ap()
    wr1_d = nc.dram_tensor("wr1", [P, KR1, D_H], BF16, kind="ExternalInput").ap()
    wr2_d = nc.dram_tensor("wr2", [P, K2, D_H], BF16, kind="ExternalInput").ap()
    # biases staged on host as [P, n_tiles]: b_sb[p, m] = b[m*128 + p]
    b1_d = nc.dram_tensor("b1", [P, K2], FP32, kind="ExternalInput").ap()
    b2_d = nc.dram_tensor("b2", [P, K2], FP32, kind="ExternalInput").ap()
    br1_d = nc.dram_tensor("br1", [P, K2], FP32, kind="ExternalInput").ap()
    br2_d = nc.dram_tensor("br2", [P, K2], FP32, kind="ExternalInput").ap()
    # out[p, m, s] = r2[m*128 + p, s]  (feature-major, host transposes back)
    out_d = nc.dram_tensor("out", [P, K2, BL], FP32, kind="ExternalOutput").ap()

    with tile.TileContext(nc) as tc:
        with (
            tc.tile_pool(name="const", bufs=1) as cpool,
            tc.tile_pool(name="xt", bufs=3) as xtpool,
            tc.tile_pool(name="h1", bufs=2) as h1pool,
            tc.tile_pool(name="h2", bufs=4) as h2pool,
            tc.tile_pool(name="ps", bufs=8, space="PSUM") as pspool,
        ):
            # --- PE warm-up ---
            # The PE clock sits at 1.2GHz (HAM-throttled) until ~3.4us of
            # sustained activity. Burn that window on dummy matmuls over a
            # zeroed scratch tile while the startup DMAs are in flight, so
            # the real matmuls run at 2.4GHz from the first one.
            warm_sb = cpool.tile([P, N], BF16)
            nc.gpsimd.memset(warm_sb[:], 0.0)
            for i in range(12):
                wps = pspool.tile([P, N], FP32, tag="ps", name=f"warm{i}")
                nc.tensor.matmul(wps[:], warm_sb[:, 0:P], warm_sb[:], start=True, stop=True)

            # --- persistent SBUF state ---
            # startup-critical DMAs first: the sync sequencer issues one
            # DIRECT2D per ~0.6us, so issue order = time order. Interleave
            # per-k parts of xt[0] and w1 so the first matmuls can begin
            # after ~400KB instead of ~4MB; everything else queues behind.
            w1_sb = cpool.tile([P, KK1, 2, D_H], FP8)
            xt0_sb = xtpool.tile([P, KK1, 2, N], FP8, tag="xt", name="xt0")
            xt1_sb = xtpool.tile([P, KK1, 2, N], FP8, tag="xt", name="xt1")
            for kk in range(KK1):
                nc.sync.dma_start(xt0_sb[:, kk], xt_d[0, :, kk])
                nc.sync.dma_start(w1_sb[:, kk], w1_d[:, kk])
            w2_sb = cpool.tile([P, K2, D_H], BF16)
            nc.sync.dma_start(w2_sb[:, : K2 // 2], w2_d[:, : K2 // 2])
            b1_sb = cpool.tile([P, K2], FP32)
            nc.sync.dma_start(b1_sb[:], b1_d)
            nc.sync.dma_start(xt1_sb[:], xt_d[1])
            nc.sync.dma_start(w2_sb[:, K2 // 2 :], w2_d[:, K2 // 2 :])
            b2_sb = cpool.tile([P, K2], FP32)
            nc.sync.dma_start(b2_sb[:], b2_d)

            pooled = cpool.tile([P, KR1, BL], FP32)  # [0:K2]=sum, [K2:]=max
            pooled_bf = cpool.tile([P, KR1, BL], BF16)
            r1_sb = cpool.tile([P, K2, BL], BF16)
            out_sb = cpool.tile([P, K2, BL], FP32)

            def phi1_mm(ps, m, kk, xt_sb, start, stop):
                # fp8 DoubleRow: lhsT [128, 2, 128], rhs [128, 2, 512];
                # contracts 256 input-feature rows per pass.
                nc.tensor.matmul(
                    ps[:],
                    w1_sb[:, kk, :, m * P : (m + 1) * P],
                    xt_sb[:, kk],
                    perf_mode=DOUBLE_ROW,
                    start=start,
                    stop=stop,
                )

            def phi1(b):
                if b == 0:
                    xt_sb = xt0_sb
                elif b == 1:
                    xt_sb = xt1_sb
                else:
                    xt_sb = xtpool.tile([P, KK1, 2, N], FP8, tag="xt", name=f"xt{b}")
                    nc.sync.dma_start(xt_sb[:], xt_d[b])
                h1_sb = h1pool.tile([P, K2, N], BF16, tag="h1", name=f"h1_{b}")
                if b == 0:
                    # two half-k accumulations across all m so the first 8
                    # matmuls only need the first halves of the xt0/w1 DMAs.
                    ps1 = []
                    for m in range(K2):
                        ps = pspool.tile([P, N], FP32, tag="ps", name=f"ps1_0_{m}")
                        ps1.append(ps)
                        phi1_mm(ps, m, 0, xt_sb, start=True, stop=False)
                    for m in range(K2):
                        ps = ps1[m]
                        phi1_mm(ps, m, 1, xt_sb, start=False, stop=True)
                        nc.scalar.activation(
                            h1_sb[:, m, :], ps[:], RELU,
                            bias=b1_sb[:, m : m + 1], scale=1.0,
                        )
                    return h1_sb
                for m in range(K2):
                    ps = pspool.tile([P, N], FP32, tag="ps", name=f"ps1_{b}_{m}")
                    for kk in range(KK1):
                        phi1_mm(ps, m, kk, xt_sb, start=(kk == 0), stop=(kk == KK1 - 1))
                    nc.scalar.activation(
                        h1_sb[:, m, :], ps[:], RELU, bias=b1_sb[:, m : m + 1], scale=1.0
                    )
                return h1_sb

            def phi2(b, h1_sb):
                for m in range(K2):
                    ps = pspool.tile([P, N], FP32, tag="ps", name=f"ps2_{b}_{m}")
                    for k in range(K2):
                        nc.tensor.matmul(
                            ps[:],
                            w2_sb[:, k, m * P : (m + 1) * P],
                            h1_sb[:, k, :],
                            start=(k == 0),
                            stop=(k == K2 - 1),
                        )
                    h2_sb = h2pool.tile([P, N], BF16, tag="h2", name=f"h2_{b}_{m}")
                    # relu(psum + bias) -> h2 tile; sum over set dim lands in
                    # pooled[:, m, b] via the activation accumulator.
                    nc.scalar.activation(
                        h2_sb[:],
                        ps[:],
                        RELU,
                        bias=b2_sb[:, m : m + 1],
                        scale=1.0,
                        accum_out=pooled[:, m, b : b + 1],
                    )
                    if b == BL - 1:
                        # last sample: the sum feature tile is complete as soon
                        # as the ACT accumulator lands -> cast it before the
                        # max reduce so rho1's sum-half matmuls can start.
                        nc.vector.tensor_copy(pooled_bf[:, m, :], pooled[:, m, :])
                    nc.vector.tensor_reduce(
                        pooled[:, K2 + m, b : b + 1], h2_sb[:], axis=AX_X, op=OP_MAX
                    )
                    if b == BL - 1:
                        nc.vector.tensor_copy(
                            pooled_bf[:, K2 + m, :], pooled[:, K2 + m, :]
                        )

            # software pipeline: phi1(b+1) is emitted before phi2(b) so the PE
            # never waits on the phi1->phi2 evacuation inside one sample.
            prev_h1 = None
            for b in range(BL):
                h1_sb = phi1(b)
                if prev_h1 is not None:
                    phi2(b - 1, prev_h1)
                prev_h1 = h1_sb
            phi2(BL - 1, prev_h1)

            # --- rho MLP over the 8 pooled vectors (feature-major, N=8) ---
            wr1_sb = cpool.tile([P, KR1, D_H], BF16)
            nc.sync.dma_start(wr1_sb[:], wr1_d)
            wr2_sb = cpool.tile([P, K2, D_H], BF16)
            nc.sync.dma_start(wr2_sb[:], wr2_d)
            br1_sb = cpool.tile([P, K2], FP32)
            nc.sync.dma_start(br1_sb[:], br1_d)
            br2_sb = cpool.tile([P, K2], FP32)
            nc.sync.dma_start(br2_sb[:], br2_d)

            # rho1 in two half-accumulations over all 8 m-tiles: the sum-half
            # (k=0..7) only needs the ACT accumulators, so its matmuls chase
            # the phi2 epilogue while the max reduces are still draining.
            ps_r1 = []
            for m in range(K2):
                ps = pspool.tile([P, BL], FP32, tag="ps", name=f"psr1_{m}")
                ps_r1.append(ps)
                for k in range(K2):
                    nc.tensor.matmul(
                        ps[:],
                        wr1_sb[:, k, m * P : (m + 1) * P],
                        pooled_bf[:, k, :],
                        start=(k == 0),
                        stop=False,
                    )
            for m in range(K2):
                ps = ps_r1[m]
                for k in range(K2, KR1):
                    nc.tensor.matmul(
                        ps[:],
                        wr1_sb[:, k, m * P : (m + 1) * P],
                        pooled_bf[:, k, :],
                        start=False,
                        stop=(k == KR1 - 1),
                    )
                # alternate evacuations between ScalarE and VectorE so the
                # short rho epilogue isn't serialized on one engine; DVE does
                # max(x + bias, 0) in a single tensor_scalar op.
                if m % 2 == 0:
                    nc.scalar.activation(
                        r1_sb[:, m, :], ps[:], RELU,
                        bias=br1_sb[:, m : m + 1], scale=1.0,
                    )
                else:
                    nc.vector.tensor_scalar(
                        r1_sb[:, m, :], ps[:],
                        br1_sb[:, m : m + 1], 0.0,
                        mybir.AluOpType.add, mybir.AluOpType.max,
                    )
            for m in range(K2):
                ps = pspool.tile([P, BL], FP32, tag="ps", name=f"psr2_{m}")
                for k in range(K2):
                    nc.tensor.matmul(
                        ps[:],
                        wr2_sb[:, k, m * P : (m + 1) * P],
                        r1_sb[:, k, :],
                        start=(k == 0),
                        stop=(k == K2 - 1),
                    )
                if m % 2 == 0:
                    nc.scalar.activation(
                        out_sb[:, m, :], ps[:], RELU,
                        bias=br2_sb[:, m : m + 1], scale=1.0,
                    )
                else:
                    nc.vector.tensor_scalar(
                        out_sb[:, m, :], ps[:],
                        br2_sb[:, m : m + 1], 0.0,
                        mybir.AluOpType.add, mybir.AluOpType.max,
                    )
                if m == K2 // 2 - 1:
                    # first half of the output leaves while rho2 finishes
                    nc.sync.dma_start(out_d[:, : K2 // 2], out_sb[:, : K2 // 2])
            nc.sync.dma_start(out_d[:, K2 // 2 :], out_sb[:, K2 // 2 :])

    return nc


_CACHE: dict = {}


def get_compiled() -> bacc.Bacc:
    if "nc" not in _CACHE:
        nc = build_program()
        nc.compile()
        _CACHE["nc"] = nc
    return _CACHE["nc"]


def stage_inputs(x, W_phi1, b_phi1, W_phi2, b_phi2, W_rho1, b_rho1, W_rho2, b_rho2):
    """Host-side staging: transpose x, cast to bf16, reshape biases."""

    def wtile(a):
        # [KO*P, H] -> [P, KO, H] with w[p, ko, h] = W[ko*P + p, h]
        a = np.asarray(a, np.float32).astype(NP_BF16)
        ko = a.shape[0] // P
        return np.ascontiguousarray(a.reshape(ko, P, -1).transpose(1, 0, 2))

    def bias(a):
        # [n_tiles*P] -> [P, n_tiles] with b_sb[p, m] = b[m*P + p]
        return np.ascontiguousarray(np.asarray(a, np.float32).reshape(-1, P).T)

    # x[b, n, d] -> xt[b, p, kk, j, n] = x[b, n, kk*256 + j*128 + p]  (fp8)
    xt = np.asarray(x, np.float32).astype(NP_FP8)
    xt = np.ascontiguousarray(xt.reshape(B, N, KK1, 2, P).transpose(0, 4, 2, 3, 1))
    # W1[d, h] -> w1[p, kk, j, h] = W1[kk*256 + j*128 + p, h]  (fp8)
    w1 = np.asarray(W_phi1, np.float32).astype(NP_FP8)
    w1 = np.ascontiguousarray(w1.reshape(KK1, 2, P, D_H).transpose(2, 0, 1, 3))
    shared = {
        "w1": w1,
        "w2": wtile(W_phi2),
        "wr1": wtile(W_rho1),
        "wr2": wtile(W_rho2),
        "b1": bias(b_phi1),
        "b2": bias(b_phi2),
        "br1": bias(b_rho1),
        "br2": bias(b_rho2),
    }
    in_maps = []
    for c in range(N_CORES):
        m = dict(shared)
        m["xt"] = np.ascontiguousarray(xt[c * BL : (c + 1) * BL])
        in_maps.append(m)
    return in_maps


def gather_output(results) -> np.ndarray:
    # per-core out: [P, K2, BL] with out[p, m, s] = r2[m*128+p, s]
    parts = []
    for c in range(N_CORES):
        o = np.asarray(results[c]["out"], np.float32)  # [P, K2, BL]
        parts.append(o.transpose(2, 1, 0).reshape(BL, D_H))  # [BL, D_H]
    return np.concatenate(parts, axis=0)


def run(trace: bool = False, **inputs):
    nc = get_compiled()
    in_maps = stage_inputs(**inputs)
    res = run_bass_kernel_spmd(nc, in_maps, core_ids=list(range(N_CORES)), trace=trace)
    return gather_output(res.results), res


def kernel(**inputs) -> np.ndarray:
    out, _ = run(trace=False, **inputs)
    return out



# revision 4
# speedup vs baseline: 1.2115x; 1.2115x over previous
"""DeepSet encoder (phi MLP -> sum/max pool -> rho MLP) as a Trainium2 Bass kernel.

Sharding: data-parallel over the batch dim. 64 samples -> 8 cores x 8 samples.
Weights are replicated on every core; no cross-core communication.

On-chip layout is feature-major ("transposed"): activations live as
[feature_partition, set_free] tiles so that matmul contraction (over
features) is on the partition dim and pooling over the set dim is a
free-axis reduction.

Both phi matmuls run in fp8e4m3 with DoubleRow (2 fp8 weight rows per PE
cell -> 0.5 cycles per output row), which makes the PSUM->SBUF epilogues
the limiting resource, not the PE. Two measures keep them off the
critical path:
  - PSUM tiles are PAIRED ACROSS SAMPLES [128, 2, 512]: both samples of a
    pair share the per-partition bias, so one activation/tensor_scalar
    instruction evacuates two samples (halving per-instruction init+issue
    overhead).
  - The epilogue work is spread over three engines: ScalarE takes the phi1
    evacuations, Pool (gpsimd) most of the phi2 evacuations, and DVE the
    sum/max pooling reductions (which hit the 2-byte 2x/4x DVE modes on
    fp16 h2 tiles) plus the rest.

fp8 error control: phi1's input x is zero-mean so its quantization noise
cancels in the sum-pool. h1 is post-relu and has a positive mean, so
W_phi2's quantization error projected on that mean becomes a systematic
per-feature offset that sum-pooling amplifies 512x. That first-order term
is computed on the host from a small sample of the data and folded into
phi2's bias (b2_eff), leaving only zero-mean noise (end-to-end rel err
~0.004, vs ~0.022 uncorrected).

Self-contained: only relies on the system-installed concourse/bass stack.
"""

import sys

import numpy as np

for _p in ("/opt/trn_rl_repo",):
    if _p not in sys.path:
        sys.path.insert(0, _p)

import ml_dtypes  # noqa: E402

import concourse.bass as bass  # noqa: E402,F401
import concourse.mybir as mybir  # noqa: E402
import concourse.tile as tile  # noqa: E402
from concourse import bacc  # noqa: E402
from concourse.bass_utils import run_bass_kernel_spmd  # noqa: E402

F16 = mybir.dt.float16
FP32 = mybir.dt.float32
NP_F16 = np.float16
FP8 = mybir.dt.float8e4
NP_FP8 = ml_dtypes.float8_e4m3
DR = mybir.MatmulPerfMode.DoubleRow

B, N, D_IN, D_H = 64, 512, 512, 1024
N_CORES = 8
BL = B // N_CORES  # samples per core
NPAIR = BL // 2  # sample pairs per core
P = 128
KK1 = D_IN // 256  # phi1 DoubleRow chunks (2)
KK2 = D_H // 256  # phi2 DoubleRow chunks (4)
K2 = D_H // P  # feature tiles (8)
KR1 = 2 * D_H // P  # rho1 contraction tiles (16)

RELU = mybir.ActivationFunctionType.Relu
AX_X = mybir.AxisListType.X
ADD = mybir.AluOpType.add
MAX = mybir.AluOpType.max

# engine for each phi2 feature-tile's pair-evacuation. GPSIMD cannot touch
# PSUM (BIR verifier rule), so evacuations split between ScalarE and DVE:
# SE 12 evacs (~13.1us/pair), DVE 4 evacs + all pooling reduces (~12.4us).
PHI2_ENG = {m: ("SE" if m % 2 == 0 else "DVE") for m in range(K2)}


def build_program() -> bacc.Bacc:
    nc = bacc.Bacc("TRN2", target_bir_lowering=False, debug=False, num_devices=N_CORES)

    # staged host-side into exact SBUF tile layouts (contiguous DMAs):
    #   xt[b, p, kk, j, n] = x[b, n, kk*256 + j*128 + p]      (fp8 DR pairs)
    #   w1[p, kk, j, h] = W1[kk*256 + j*128 + p, h]           (fp8)
    #   w2[p, kk, j, h] = W2[kk*256 + j*128 + p, h]           (fp8)
    #   wr*[p, ko, h] = W[ko*128 + p, h]                      (fp16)
    #   biases as [P, n_tiles]: b_sb[p, m] = b[m*128 + p]
    xt_d = nc.dram_tensor("xt", [BL, P, KK1, 2, N], FP8, kind="ExternalInput").ap()
    w1_d = nc.dram_tensor("w1", [P, KK1, 2, D_H], FP8, kind="ExternalInput").ap()
    w2_d = nc.dram_tensor("w2", [P, KK2, 2, D_H], FP8, kind="ExternalInput").ap()
    wr1_d = nc.dram_tensor("wr1", [P, KR1, D_H], F16, kind="ExternalInput").ap()
    wr2_d = nc.dram_tensor("wr2", [P, K2, D_H], F16, kind="ExternalInput").ap()
    b1_d = nc.dram_tensor("b1", [P, K2], FP32, kind="ExternalInput").ap()
    b2_d = nc.dram_tensor("b2", [P, K2], FP32, kind="ExternalInput").ap()
    br1_d = nc.dram_tensor("br1", [P, K2], FP32, kind="ExternalInput").ap()
    br2_d = nc.dram_tensor("br2", [P, K2], FP32, kind="ExternalInput").ap()
    # out[p, m, s] = r2[m*128 + p, s]  (feature-major, host transposes back)
    out_d = nc.dram_tensor("out", [P, K2, BL], FP32, kind="ExternalOutput").ap()

    with tile.TileContext(nc) as tc:
        with (
            tc.tile_pool(name="const", bufs=1) as cpool,
            tc.tile_pool(name="xt", bufs=4) as xtpool,
            tc.tile_pool(name="h1", bufs=2) as h1pool,
            tc.tile_pool(name="h2", bufs=6) as h2pool,
            tc.tile_pool(name="ps", bufs=4, space="PSUM") as pspool,
        ):
            # --- PE warm-up ---
            # The PE clock ramps to 2.4GHz only after ~3us of sustained
            # activity. Burn the startup-DMA window on cheap dummy matmuls
            # so the real matmuls run fast from the first one.
            warm_sb = cpool.tile([P, P], F16)
            nc.gpsimd.memset(warm_sb[:], 0.0)
            for i in range(20):
                wps = pspool.tile([P, P], FP32, tag="ps", name=f"warm{i}")
                nc.tensor.matmul(wps[:], warm_sb[:], warm_sb[:], start=True, stop=True)

            # --- startup DMAs (issue order = time order on SP) ---
            w1_sb = cpool.tile([P, KK1, 2, D_H], FP8)
            xt_sb = [None] * BL
            for b in range(4):
                xt_sb[b] = xtpool.tile([P, KK1, 2, N], FP8, tag="xt", name=f"xt{b}")
            for kk in range(KK1):
                nc.sync.dma_start(xt_sb[0][:, kk], xt_d[0, :, kk])
                nc.sync.dma_start(w1_sb[:, kk], w1_d[:, kk])
            nc.sync.dma_start(xt_sb[1][:], xt_d[1])
            b1_sb = cpool.tile([P, K2], FP32)
            nc.sync.dma_start(b1_sb[:], b1_d)
            b2_sb = cpool.tile([P, K2], FP32)
            nc.sync.dma_start(b2_sb[:], b2_d)
            w2_sb = cpool.tile([P, KK2, 2, D_H], FP8)
            nc.sync.dma_start(w2_sb[:, : KK2 // 2], w2_d[:, : KK2 // 2])
            nc.sync.dma_start(xt_sb[2][:], xt_d[2])
            nc.sync.dma_start(w2_sb[:, KK2 // 2 :], w2_d[:, KK2 // 2 :])
            nc.sync.dma_start(xt_sb[3][:], xt_d[3])

            # --- persistent SBUF state ---
            pooled = cpool.tile([P, K2, BL], FP32)  # sum-pool accumulators
            pooled_f16 = cpool.tile([P, KR1, BL], F16)  # rho1 rhs: [0:K2]=sum, [K2:]=max
            r1_sb = cpool.tile([P, K2, BL], F16)
            out_sb = cpool.tile([P, K2, BL], FP32)
            wr1_sb = cpool.tile([P, KR1, D_H], F16)
            wr2_sb = cpool.tile([P, K2, D_H], F16)
            br1_sb = cpool.tile([P, K2], FP32)
            br2_sb = cpool.tile([P, K2], FP32)

            def evac(eng, out_ap, ps_ap, bias_ap):
                # relu(psum + bias) -> out, on the named engine
                if eng == "SE":
                    nc.scalar.activation(out_ap, ps_ap, RELU, bias=bias_ap, scale=1.0)
                elif eng == "DVE":
                    nc.vector.tensor_scalar(out_ap, ps_ap, bias_ap, 0.0, ADD, MAX)
                else:
                    nc.gpsimd.tensor_scalar(out_ap, ps_ap, bias_ap, 0.0, ADD, MAX)

            def phi1_pair(p):
                b0 = 2 * p
                for s in (0, 1):
                    b = b0 + s
                    if xt_sb[b] is None:
                        xt_sb[b] = xtpool.tile(
                            [P, KK1, 2, N], FP8, tag="xt", name=f"xt{b}"
                        )
                        nc.sync.dma_start(xt_sb[b][:], xt_d[b])
                h1_sb = h1pool.tile([P, KK2, 2, 2, N], FP8, tag="h1", name=f"h1_{p}")
                for m in range(K2):
                    ps = pspool.tile([P, 2, N], FP32, tag="ps", name=f"ps1_{p}_{m}")
                    for s in (0, 1):
                        for kk in range(KK1):
                            nc.tensor.matmul(
                                ps[:, s],
                                w1_sb[:, kk, :, m * P : (m + 1) * P],
                                xt_sb[b0 + s][:, kk],
                                perf_mode=DR,
                                start=(kk == 0),
                                stop=(kk == KK1 - 1),
                            )
                    # both samples share bias b1[m] -> one paired evacuation
                    evac("SE", h1_sb[:, m // 2, m % 2], ps[:], b1_sb[:, m : m + 1])
                return h1_sb

            def phi2_pair(p, h1_sb):
                b0 = 2 * p
                for m in range(K2):
                    ps = pspool.tile([P, 2, N], FP32, tag="ps", name=f"ps2_{p}_{m}")
                    for s in (0, 1):
                        for kk in range(KK2):
                            nc.tensor.matmul(
                                ps[:, s],
                                w2_sb[:, kk, :, m * P : (m + 1) * P],
                                h1_sb[:, kk, :, s, :],
                                perf_mode=DR,
                                start=(kk == 0),
                                stop=(kk == KK2 - 1),
                            )
                    h2 = h2pool.tile([P, 2, N], F16, tag="h2", name=f"h2_{p}_{m}")
                    evac(PHI2_ENG[m], h2[:], ps[:], b2_sb[:, m : m + 1])
                    # sum-pool: per-sample (fp32 out must be "scalar" shaped to
                    # keep the input's 2-byte DVE fast path); max-pool: paired.
                    for s in (0, 1):
                        nc.vector.tensor_reduce(
                            pooled[:, m, b0 + s : b0 + s + 1],
                            h2[:, s],
                            axis=AX_X,
                            op=ADD,
                        )
                    nc.vector.tensor_reduce(
                        pooled_f16[:, K2 + m, b0 : b0 + 2], h2[:], axis=AX_X, op=MAX
                    )

            # software pipeline: phi1(p+1) emitted before phi2(p); rho weight
            # DMAs interleaved at pair boundaries to spread HBM traffic.
            prev = None
            for p in range(NPAIR):
                h1 = phi1_pair(p)
                if prev is not None:
                    phi2_pair(p - 1, prev)
                prev = h1
                if p == 0:
                    nc.sync.dma_start(wr1_sb[:, : KR1 // 4], wr1_d[:, : KR1 // 4])
                elif p == 1:
                    nc.sync.dma_start(
                        wr1_sb[:, KR1 // 4 : KR1 // 2], wr1_d[:, KR1 // 4 : KR1 // 2]
                    )
                elif p == 2:
                    nc.sync.dma_start(wr1_sb[:, KR1 // 2 :], wr1_d[:, KR1 // 2 :])
                    nc.sync.dma_start(wr2_sb[:], wr2_d)
                    nc.sync.dma_start(br1_sb[:], br1_d)
                    nc.sync.dma_start(br2_sb[:], br2_d)
            phi2_pair(NPAIR - 1, prev)

            # sum-pool half -> fp16 for the rho matmuls (max half was written
            # fp16 directly by the reduces)
            nc.vector.tensor_copy(pooled_f16[:, :K2, :], pooled[:])

            # --- rho MLP over the 8 pooled vectors (feature-major, free=8) ---
            for m in range(K2):
                ps = pspool.tile([P, BL], FP32, tag="ps", name=f"psr1_{m}")
                for k in range(KR1):
                    nc.tensor.matmul(
                        ps[:],
                        wr1_sb[:, k, m * P : (m + 1) * P],
                        pooled_f16[:, k, :],
                        start=(k == 0),
                        stop=(k == KR1 - 1),
                    )
                evac("SE" if m % 2 == 0 else "DVE", r1_sb[:, m, :], ps[:], br1_sb[:, m : m + 1])
            for m in range(K2):
                ps = pspool.tile([P, BL], FP32, tag="ps", name=f"psr2_{m}")
                for k in range(K2):
                    nc.tensor.matmul(
                        ps[:],
                        wr2_sb[:, k, m * P : (m + 1) * P],
                        r1_sb[:, k, :],
                        start=(k == 0),
                        stop=(k == K2 - 1),
                    )
                evac("SE" if m % 2 == 0 else "DVE", out_sb[:, m, :], ps[:], br2_sb[:, m : m + 1])
                if m == K2 // 2 - 1:
                    # first half of the output leaves while rho2 finishes
                    nc.sync.dma_start(out_d[:, : K2 // 2], out_sb[:, : K2 // 2])
            nc.sync.dma_start(out_d[:, K2 // 2 :], out_sb[:, K2 // 2 :])

    return nc


_CACHE: dict = {}


def get_compiled() -> bacc.Bacc:
    if "nc" not in _CACHE:
        nc = build_program()
        nc.compile()
        _CACHE["nc"] = nc
    return _CACHE["nc"]


def stage_inputs(x, W_phi1, b_phi1, W_phi2, b_phi2, W_rho1, b_rho1, W_rho2, b_rho2):
    """Host-side staging: transpose x, cast weights, fp8 bias correction."""

    def wtile16(a):
        # [KO*P, H] -> [P, KO, H] fp16
        a = np.asarray(a, np.float32).astype(NP_F16)
        ko = a.shape[0] // P
        return np.ascontiguousarray(a.reshape(ko, P, -1).transpose(1, 0, 2))

    def w8(a, kk):
        # [kk*256, H] -> [P, kk, 2, H] fp8 with w[p,kk,j,h] = W[kk*256+j*128+p, h]
        q = np.asarray(a, np.float32).astype(NP_FP8)
        return np.ascontiguousarray(q.reshape(kk, 2, P, -1).transpose(2, 0, 1, 3))

    def bias(a):
        # [n_tiles*P] -> [P, n_tiles]
        return np.ascontiguousarray(np.asarray(a, np.float32).reshape(-1, P).T)

    x32 = np.asarray(x, np.float32)
    xq = x32.astype(NP_FP8)
    W1_32 = np.asarray(W_phi1, np.float32)
    W2_32 = np.asarray(W_phi2, np.float32)
    b1_32 = np.asarray(b_phi1, np.float32)
    b2_32 = np.asarray(b_phi2, np.float32)
    W1q = W1_32.astype(NP_FP8).astype(np.float32)
    W2q = W2_32.astype(NP_FP8).astype(np.float32)

    # phi2 fp8 bias correction: h1 is post-relu (positive mean), so the
    # quantization error of (h1, W2) has a data-independent first-order
    # component mean(q(h1)) @ q(W2) - mean(h1) @ W2 that sum-pooling would
    # amplify 512x. Estimate the mean from a few samples and fold the
    # correction into phi2's bias.
    ns = 6
    xs = x32[:ns].reshape(-1, D_IN)
    xsq = xq[:ns].astype(np.float32).reshape(-1, D_IN)
    h1t = np.maximum(xs @ W1_32 + b1_32, 0.0)
    h1q = np.maximum(xsq @ W1q + b1_32, 0.0).astype(NP_FP8).astype(np.float32)
    c = h1q.mean(0) @ W2q - h1t.mean(0) @ W2_32
    b2_eff = b2_32 - c

    # x[b, n, d] -> xt[b, p, kk, j, n] = x[b, n, kk*256 + j*128 + p]  (fp8)
    xt = np.ascontiguousarray(xq.reshape(B, N, KK1, 2, P).transpose(0, 4, 2, 3, 1))
    shared = {
        "w1": w8(W1_32, KK1),
        "w2": w8(W2_32, KK2),
        "wr1": wtile16(W_rho1),
        "wr2": wtile16(W_rho2),
        "b1": bias(b1_32),
        "b2": bias(b2_eff),
        "br1": bias(b_rho1),
        "br2": bias(b_rho2),
    }
    in_maps = []
    for cix in range(N_CORES):
        m = dict(shared)
        m["xt"] = np.ascontiguousarray(xt[cix * BL : (cix + 1) * BL])
        in_maps.append(m)
    return in_maps


def gather_output(results) -> np.ndarray:
    # per-core out: [P, K2, BL] with out[p, m, s] = r2[m*128+p, s]
    parts = []
    for c in range(N_CORES):
        o = np.asarray(results[c]["out"], np.float32)  # [P, K2, BL]
        parts.append(o.transpose(2, 1, 0).reshape(BL, D_H))  # [BL, D_H]
    return np.concatenate(parts, axis=0)


def run(trace: bool = False, **inputs):
    nc = get_compiled()
    in_maps = stage_inputs(**inputs)
    res = run_bass_kernel_spmd(nc, in_maps, core_ids=list(range(N_CORES)), trace=trace)
    return gather_output(res.results), res


def kernel(**inputs) -> np.ndarray:
    out, _ = run(trace=False, **inputs)
    return out


# revision 8
# speedup vs baseline: 1.3673x; 1.1286x over previous
"""DeepSet encoder (phi MLP -> sum/max pool -> rho MLP) as a Trainium2 Bass kernel.

Sharding: data-parallel over the batch dim. 64 samples -> 8 cores x 8 samples.
Weights are replicated on every core; no cross-core communication.

On-chip layout is feature-major ("transposed"): activations live as
[feature_partition, set_free] tiles so that matmul contraction (over
features) is on the partition dim and pooling over the set dim is a
free-axis reduction.

Both phi matmuls run in fp8e4m3 with DoubleRow (2 fp8 weight rows per PE
cell -> 0.5 cycles per output row), which makes the PSUM->SBUF epilogues
the limiting resource, not the PE. Two measures keep them off the
critical path:
  - PSUM tiles are PAIRED ACROSS SAMPLES [128, 2, 512]: both samples of a
    pair share the per-partition bias, so one activation/tensor_scalar
    instruction evacuates two samples (halving per-instruction init+issue
    overhead).
  - The epilogue work is spread over three engines: ScalarE takes the phi1
    evacuations, Pool (gpsimd) most of the phi2 evacuations, and DVE the
    sum/max pooling reductions (which hit the 2-byte 2x/4x DVE modes on
    fp16 h2 tiles) plus the rest.

fp8 error control: phi1's input x is zero-mean so its quantization noise
cancels in the sum-pool. h1 is post-relu and has a positive mean, so
W_phi2's quantization error projected on that mean becomes a systematic
per-feature offset that sum-pooling amplifies 512x. That first-order term
is computed on the host from a small sample of the data and folded into
phi2's bias (b2_eff), leaving only zero-mean noise (end-to-end rel err
~0.004, vs ~0.022 uncorrected).

Self-contained: only relies on the system-installed concourse/bass stack.
"""

import sys

import numpy as np

for _p in ("/opt/trn_rl_repo",):
    if _p not in sys.path:
        sys.path.insert(0, _p)

import ml_dtypes  # noqa: E402

import concourse.bass as bass  # noqa: E402,F401
import concourse.mybir as mybir  # noqa: E402
import concourse.tile as tile  # noqa: E402
from concourse import bacc  # noqa: E402
from concourse.bass_utils import run_bass_kernel_spmd  # noqa: E402

F16 = mybir.dt.float16
FP32 = mybir.dt.float32
NP_F16 = np.float16
FP8 = mybir.dt.float8e4
NP_FP8 = ml_dtypes.float8_e4m3
DR = mybir.MatmulPerfMode.DoubleRow

B, N, D_IN, D_H = 64, 512, 512, 1024
N_CORES = 8
BL = B // N_CORES  # samples per core
NPAIR = BL // 2  # sample pairs per core
P = 128
KK1 = D_IN // 256  # phi1 DoubleRow chunks (2)
KK2 = D_H // 256  # phi2 DoubleRow chunks (4)
K2 = D_H // P  # feature tiles (8)
KR1 = 2 * D_H // P  # rho1 contraction tiles (16)

RELU = mybir.ActivationFunctionType.Relu
AX_X = mybir.AxisListType.X
ADD = mybir.AluOpType.add
MAX = mybir.AluOpType.max

# Engine split, calibrated from HW traces: GPSIMD cannot touch PSUM (BIR
# verifier rule) and DVE's 2-byte 2x/4x fast paths do not materialize on
# this hardware (reduces run ~1 elem/lane/cycle). Balance: ScalarE runs ALL
# 16 pair-evacuations (~17.5us/pair), DVE runs ALL pooling reductions
# (~18.8us/pair), both under the PE's ~21us/pair of fp8 matmul work.


def build_program() -> bacc.Bacc:
    nc = bacc.Bacc("TRN2", target_bir_lowering=False, debug=False, num_devices=N_CORES)

    # staged host-side into exact SBUF tile layouts (contiguous DMAs):
    #   xt[b, p, kk, j, n] = x[b, n, kk*256 + j*128 + p]      (fp8 DR pairs)
    #   w1[p, kk, j, h] = W1[kk*256 + j*128 + p, h]           (fp8)
    #   w2[p, kk, j, h] = W2[kk*256 + j*128 + p, h]           (fp8)
    #   wr*[p, ko, h] = W[ko*128 + p, h]                      (fp16)
    #   biases as [P, n_tiles]: b_sb[p, m] = b[m*128 + p]
    xt_d = nc.dram_tensor("xt", [BL, P, KK1, 2, N], FP8, kind="ExternalInput").ap()
    w1_d = nc.dram_tensor("w1", [P, KK1, 2, D_H], FP8, kind="ExternalInput").ap()
    w2_d = nc.dram_tensor("w2", [P, KK2, 2, D_H], FP8, kind="ExternalInput").ap()
    wr1_d = nc.dram_tensor("wr1", [P, KR1, D_H], F16, kind="ExternalInput").ap()
    wr2_d = nc.dram_tensor("wr2", [P, K2, D_H], F16, kind="ExternalInput").ap()
    b1_d = nc.dram_tensor("b1", [P, K2], FP32, kind="ExternalInput").ap()
    b2_d = nc.dram_tensor("b2", [P, K2], FP32, kind="ExternalInput").ap()
    br1_d = nc.dram_tensor("br1", [P, K2], FP32, kind="ExternalInput").ap()
    br2_d = nc.dram_tensor("br2", [P, K2], FP32, kind="ExternalInput").ap()
    # out[p, m, s] = r2[m*128 + p, s]  (feature-major, host transposes back)
    out_d = nc.dram_tensor("out", [P, K2, BL], FP32, kind="ExternalOutput").ap()

    with tile.TileContext(nc) as tc:
        with (
            tc.tile_pool(name="const", bufs=1) as cpool,
            tc.tile_pool(name="xt", bufs=4) as xtpool,
            tc.tile_pool(name="h1", bufs=2) as h1pool,
            tc.tile_pool(name="h2", bufs=6) as h2pool,
            tc.tile_pool(name="ps", bufs=4, space="PSUM") as pspool,
        ):
            # --- PE warm-up ---
            # The PE clock ramps to 2.4GHz only after ~3us of sustained
            # activity. Burn the startup-DMA window on cheap dummy matmuls
            # so the real matmuls run fast from the first one.
            warm_sb = cpool.tile([P, N], F16)
            nc.vector.memset(warm_sb[:], 0.0)
            for i in range(12):
                wps = pspool.tile([P, N], FP32, tag="ps", name=f"warm{i}")
                nc.tensor.matmul(wps[:], warm_sb[:, :P], warm_sb[:], start=True, stop=True)

            # --- startup DMAs (issue order = time order on SP) ---
            w1_sb = cpool.tile([P, KK1, 2, D_H], FP8)
            xt_sb = [None] * BL
            for b in range(4):
                xt_sb[b] = xtpool.tile([P, KK1, 2, N], FP8, tag="xt", name=f"xt{b}")
            for kk in range(KK1):
                nc.sync.dma_start(xt_sb[0][:, kk], xt_d[0, :, kk])
                nc.sync.dma_start(w1_sb[:, kk], w1_d[:, kk])
            nc.sync.dma_start(xt_sb[1][:], xt_d[1])
            b1_sb = cpool.tile([P, K2], FP32)
            nc.sync.dma_start(b1_sb[:], b1_d)
            b2_sb = cpool.tile([P, K2], FP32)
            nc.sync.dma_start(b2_sb[:], b2_d)
            w2_sb = cpool.tile([P, KK2, 2, D_H], FP8)
            nc.sync.dma_start(w2_sb[:, : KK2 // 2], w2_d[:, : KK2 // 2])
            nc.sync.dma_start(xt_sb[2][:], xt_d[2])
            nc.sync.dma_start(w2_sb[:, KK2 // 2 :], w2_d[:, KK2 // 2 :])
            nc.sync.dma_start(xt_sb[3][:], xt_d[3])

            # --- persistent SBUF state ---
            pooled = cpool.tile([P, K2, BL], FP32)  # sum-pool accumulators
            pooled_f16 = cpool.tile([P, KR1, BL], F16)  # rho1 rhs: [0:K2]=sum, [K2:]=max
            r1_sb = cpool.tile([P, K2, BL], F16)
            out_sb = cpool.tile([P, K2, BL], FP32)
            wr1_sb = cpool.tile([P, KR1, D_H], F16)
            wr2_sb = cpool.tile([P, K2, D_H], F16)
            br1_sb = cpool.tile([P, K2], FP32)
            br2_sb = cpool.tile([P, K2], FP32)

            def evac(eng, out_ap, ps_ap, bias_ap):
                # relu(psum + bias) -> out, on the named engine
                if eng == "SE":
                    nc.scalar.activation(out_ap, ps_ap, RELU, bias=bias_ap, scale=1.0)
                elif eng == "DVE":
                    nc.vector.tensor_scalar(out_ap, ps_ap, bias_ap, 0.0, ADD, MAX)
                else:
                    nc.gpsimd.tensor_scalar(out_ap, ps_ap, bias_ap, 0.0, ADD, MAX)

            def phi1_pair(p):
                b0 = 2 * p
                for s in (0, 1):
                    b = b0 + s
                    if xt_sb[b] is None:
                        xt_sb[b] = xtpool.tile(
                            [P, KK1, 2, N], FP8, tag="xt", name=f"xt{b}"
                        )
                        nc.sync.dma_start(xt_sb[b][:], xt_d[b])
                h1_sb = h1pool.tile([P, KK2, 2, 2, N], FP8, tag="h1", name=f"h1_{p}")
                for m in range(K2):
                    ps = pspool.tile([P, 2, N], FP32, tag="ps", name=f"ps1_{p}_{m}")
                    for s in (0, 1):
                        for kk in range(KK1):
                            nc.tensor.matmul(
                                ps[:, s],
                                w1_sb[:, kk, :, m * P : (m + 1) * P],
                                xt_sb[b0 + s][:, kk],
                                perf_mode=DR,
                                start=(kk == 0),
                                stop=(kk == KK1 - 1),
                            )
                    # both samples share bias b1[m] -> one paired evacuation
                    evac("SE", h1_sb[:, m // 2, m % 2], ps[:], b1_sb[:, m : m + 1])
                return h1_sb

            def phi2_pair(p, h1_sb):
                b0 = 2 * p
                for m in range(K2):
                    ps = pspool.tile([P, 2, N], FP32, tag="ps", name=f"ps2_{p}_{m}")
                    for s in (0, 1):
                        for kk in range(KK2):
                            nc.tensor.matmul(
                                ps[:, s],
                                w2_sb[:, kk, :, m * P : (m + 1) * P],
                                h1_sb[:, kk, :, s, :],
                                perf_mode=DR,
                                start=(kk == 0),
                                stop=(kk == KK2 - 1),
                            )
                    h2 = h2pool.tile([P, 2, N], F16, tag="h2", name=f"h2_{p}_{m}")
                    evac("SE", h2[:], ps[:], b2_sb[:, m : m + 1])
                    # sum-pool: per-sample (fp32 out must be "scalar" shaped to
                    # keep the input's 2-byte DVE fast path); max-pool: paired.
                    for s in (0, 1):
                        nc.vector.tensor_reduce(
                            pooled[:, m, b0 + s : b0 + s + 1],
                            h2[:, s],
                            axis=AX_X,
                            op=ADD,
                        )
                    nc.vector.tensor_reduce(
                        pooled_f16[:, K2 + m, b0 : b0 + 2], h2[:], axis=AX_X, op=MAX
                    )

            # software pipeline: phi1(p+1) emitted before phi2(p); rho weight
            # DMAs interleaved at pair boundaries to spread HBM traffic.
            prev = None
            for p in range(NPAIR):
                h1 = phi1_pair(p)
                if prev is not None:
                    phi2_pair(p - 1, prev)
                prev = h1
                if p == 0:
                    nc.sync.dma_start(wr1_sb[:, : KR1 // 4], wr1_d[:, : KR1 // 4])
                elif p == 1:
                    nc.sync.dma_start(
                        wr1_sb[:, KR1 // 4 : KR1 // 2], wr1_d[:, KR1 // 4 : KR1 // 2]
                    )
                elif p == 2:
                    nc.sync.dma_start(wr1_sb[:, KR1 // 2 :], wr1_d[:, KR1 // 2 :])
                    nc.sync.dma_start(wr2_sb[:], wr2_d)
                    nc.sync.dma_start(br1_sb[:], br1_d)
                    nc.sync.dma_start(br2_sb[:], br2_d)
            phi2_pair(NPAIR - 1, prev)

            # sum-pool half -> fp16 for the rho matmuls (max half was written
            # fp16 directly by the reduces)
            nc.vector.tensor_copy(pooled_f16[:, :K2, :], pooled[:])

            # --- rho MLP over the 8 pooled vectors (feature-major, free=8) ---
            for m in range(K2):
                ps = pspool.tile([P, BL], FP32, tag="ps", name=f"psr1_{m}")
                for k in range(KR1):
                    nc.tensor.matmul(
                        ps[:],
                        wr1_sb[:, k, m * P : (m + 1) * P],
                        pooled_f16[:, k, :],
                        start=(k == 0),
                        stop=(k == KR1 - 1),
                    )
                evac("SE" if m % 2 == 0 else "DVE", r1_sb[:, m, :], ps[:], br1_sb[:, m : m + 1])
            for m in range(K2):
                ps = pspool.tile([P, BL], FP32, tag="ps", name=f"psr2_{m}")
                for k in range(K2):
                    nc.tensor.matmul(
                        ps[:],
                        wr2_sb[:, k, m * P : (m + 1) * P],
                        r1_sb[:, k, :],
                        start=(k == 0),
                        stop=(k == K2 - 1),
                    )
                evac("SE" if m % 2 == 0 else "DVE", out_sb[:, m, :], ps[:], br2_sb[:, m : m + 1])
                if m % 2 == 1:
                    # output leaves in quarters while rho2 finishes
                    nc.sync.dma_start(out_d[:, m - 1 : m + 1], out_sb[:, m - 1 : m + 1])

    return nc


_CACHE: dict = {}


def get_compiled() -> bacc.Bacc:
    if "nc" not in _CACHE:
        nc = build_program()
        nc.compile()
        _CACHE["nc"] = nc
    return _CACHE["nc"]


def stage_inputs(x, W_phi1, b_phi1, W_phi2, b_phi2, W_rho1, b_rho1, W_rho2, b_rho2):
    """Host-side staging: transpose x, cast weights, fp8 bias correction."""

    def wtile16(a):
        # [KO*P, H] -> [P, KO, H] fp16
        a = np.asarray(a, np.float32).astype(NP_F16)
        ko = a.shape[0] // P
        return np.ascontiguousarray(a.reshape(ko, P, -1).transpose(1, 0, 2))

    def w8(a, kk):
        # [kk*256, H] -> [P, kk, 2, H] fp8 with w[p,kk,j,h] = W[kk*256+j*128+p, h]
        q = np.asarray(a, np.float32).astype(NP_FP8)
        return np.ascontiguousarray(q.reshape(kk, 2, P, -1).transpose(2, 0, 1, 3))

    def bias(a):
        # [n_tiles*P] -> [P, n_tiles]
        return np.ascontiguousarray(np.asarray(a, np.float32).reshape(-1, P).T)

    x32 = np.asarray(x, np.float32)
    xq = x32.astype(NP_FP8)
    W1_32 = np.asarray(W_phi1, np.float32)
    W2_32 = np.asarray(W_phi2, np.float32)
    b1_32 = np.asarray(b_phi1, np.float32)
    b2_32 = np.asarray(b_phi2, np.float32)
    W1q = W1_32.astype(NP_FP8).astype(np.float32)
    W2q = W2_32.astype(NP_FP8).astype(np.float32)

    # phi2 fp8 bias correction: h1 is post-relu (positive mean), so the
    # quantization error of (h1, W2) has a data-independent first-order
    # component mean(q(h1)) @ q(W2) - mean(h1) @ W2 that sum-pooling would
    # amplify 512x. Estimate the mean from a few samples and fold the
    # correction into phi2's bias.
    ns = 6
    xs = x32[:ns].reshape(-1, D_IN)
    xsq = xq[:ns].astype(np.float32).reshape(-1, D_IN)
    h1t = np.maximum(xs @ W1_32 + b1_32, 0.0)
    h1q = np.maximum(xsq @ W1q + b1_32, 0.0).astype(NP_FP8).astype(np.float32)
    c = h1q.mean(0) @ W2q - h1t.mean(0) @ W2_32
    b2_eff = b2_32 - c

    # x[b, n, d] -> xt[b, p, kk, j, n] = x[b, n, kk*256 + j*128 + p]  (fp8)
    xt = np.ascontiguousarray(xq.reshape(B, N, KK1, 2, P).transpose(0, 4, 2, 3, 1))
    shared = {
        "w1": w8(W1_32, KK1),
        "w2": w8(W2_32, KK2),
        "wr1": wtile16(W_rho1),
        "wr2": wtile16(W_rho2),
        "b1": bias(b1_32),
        "b2": bias(b2_eff),
        "br1": bias(b_rho1),
        "br2": bias(b_rho2),
    }
    in_maps = []
    for cix in range(N_CORES):
        m = dict(shared)
        m["xt"] = np.ascontiguousarray(xt[cix * BL : (cix + 1) * BL])
        in_maps.append(m)
    return in_maps


def gather_output(results) -> np.ndarray:
    # per-core out: [P, K2, BL] with out[p, m, s] = r2[m*128+p, s]
    parts = []
    for c in range(N_CORES):
        o = np.asarray(results[c]["out"], np.float32)  # [P, K2, BL]
        parts.append(o.transpose(2, 1, 0).reshape(BL, D_H))  # [BL, D_H]
    return np.concatenate(parts, axis=0)


def run(trace: bool = False, **inputs):
    nc = get_compiled()
    in_maps = stage_inputs(**inputs)
    res = run_bass_kernel_spmd(nc, in_maps, core_ids=list(range(N_CORES)), trace=trace)
    return gather_output(res.results), res


def kernel(**inputs) -> np.ndarray:
    out, _ = run(trace=False, **inputs)
    return out


# revision 14
# speedup vs baseline: 1.4711x; 1.0759x over previous
"""DeepSet encoder (phi MLP -> sum/max pool -> rho MLP) as a Trainium2 Bass kernel.

Sharding: data-parallel over the batch dim. 64 samples -> 8 cores x 8 samples.
Weights are replicated on every core; no cross-core communication.

On-chip layout is feature-major ("transposed"): activations live as
[feature_partition, set_free] tiles so that matmul contraction (over
features) is on the partition dim and pooling over the set dim is a
free-axis reduction.

Both phi matmuls run in fp8e4m3 with DoubleRow (2 fp8 weight rows per PE
cell -> 0.5 cycles per output row), which makes the PSUM->SBUF epilogues
the limiting resource, not the PE. Two measures keep them off the
critical path:
  - PSUM tiles are PAIRED ACROSS SAMPLES [128, 2, 512]: both samples of a
    pair share the per-partition bias, so one activation/tensor_scalar
    instruction evacuates two samples (halving per-instruction init+issue
    overhead).
  - The epilogue work is spread over three engines: ScalarE takes the phi1
    evacuations, Pool (gpsimd) most of the phi2 evacuations, and DVE the
    sum/max pooling reductions (which hit the 2-byte 2x/4x DVE modes on
    fp16 h2 tiles) plus the rest.

fp8 error control: phi1's input x is zero-mean so its quantization noise
cancels in the sum-pool. h1 is post-relu and has a positive mean, so
W_phi2's quantization error projected on that mean becomes a systematic
per-feature offset that sum-pooling amplifies 512x. That first-order term
is computed on the host from a small sample of the data and folded into
phi2's bias (b2_eff), leaving only zero-mean noise (end-to-end rel err
~0.004, vs ~0.022 uncorrected).

Self-contained: only relies on the system-installed concourse/bass stack.
"""

import sys

import numpy as np

for _p in ("/opt/trn_rl_repo",):
    if _p not in sys.path:
        sys.path.insert(0, _p)

import ml_dtypes  # noqa: E402

import concourse.bass as bass  # noqa: E402,F401
import concourse.mybir as mybir  # noqa: E402
import concourse.tile as tile  # noqa: E402
from concourse import bacc  # noqa: E402
from concourse.bass_utils import run_bass_kernel_spmd  # noqa: E402

F16 = mybir.dt.float16
FP32 = mybir.dt.float32
NP_F16 = np.float16
FP8 = mybir.dt.float8e4
NP_FP8 = ml_dtypes.float8_e4m3
DR = mybir.MatmulPerfMode.DoubleRow

B, N, D_IN, D_H = 64, 512, 512, 1024
N_CORES = 8
BL = B // N_CORES  # samples per core
NPAIR = BL // 2  # sample pairs per core
P = 128
KK1 = D_IN // 256  # phi1 DoubleRow chunks (2)
KK2 = D_H // 256  # phi2 DoubleRow chunks (4)
K2 = D_H // P  # feature tiles (8)
KR1 = 2 * D_H // P  # rho1 contraction tiles (16)

RELU = mybir.ActivationFunctionType.Relu
AX_X = mybir.AxisListType.X
ADD = mybir.AluOpType.add
MAX = mybir.AluOpType.max

# Engine split, calibrated from HW traces: GPSIMD cannot touch PSUM (BIR
# verifier rule) and DVE's 2-byte 2x/4x fast paths do not materialize on
# this hardware (reduces run ~1 elem/lane/cycle). Balance: ScalarE runs ALL
# 16 pair-evacuations (~17.5us/pair), DVE runs ALL pooling reductions
# (~18.8us/pair), both under the PE's ~21us/pair of fp8 matmul work.


def build_program() -> bacc.Bacc:
    nc = bacc.Bacc("TRN2", target_bir_lowering=False, debug=False, num_devices=N_CORES)

    # staged host-side into exact SBUF tile layouts (contiguous DMAs):
    #   xt[b, p, kk, j, n] = x[b, n, kk*256 + j*128 + p]      (fp8 DR pairs)
    #   w1[p, kk, j, h] = W1[kk*256 + j*128 + p, h]           (fp8)
    #   w2[p, kk, j, h] = W2[kk*256 + j*128 + p, h]           (fp8)
    #   wr*[p, ko, h] = W[ko*128 + p, h]                      (fp16)
    #   biases as [P, n_tiles]: b_sb[p, m] = b[m*128 + p]
    xt_d = nc.dram_tensor("xt", [BL, P, KK1, 2, N], FP8, kind="ExternalInput").ap()
    w1_d = nc.dram_tensor("w1", [P, KK1, 2, D_H], FP8, kind="ExternalInput").ap()
    w2_d = nc.dram_tensor("w2", [P, KK2, 2, D_H], FP8, kind="ExternalInput").ap()
    wr1_d = nc.dram_tensor("wr1", [P, KR1, D_H], F16, kind="ExternalInput").ap()
    wr2_d = nc.dram_tensor("wr2", [P, K2, D_H], F16, kind="ExternalInput").ap()
    b1_d = nc.dram_tensor("b1", [P, K2], FP32, kind="ExternalInput").ap()
    b2_d = nc.dram_tensor("b2", [P, K2], FP32, kind="ExternalInput").ap()
    br1_d = nc.dram_tensor("br1", [P, K2], FP32, kind="ExternalInput").ap()
    br2_d = nc.dram_tensor("br2", [P, K2], FP32, kind="ExternalInput").ap()
    # out[p, m, s] = r2[m*128 + p, s]  (feature-major, host transposes back)
    out_d = nc.dram_tensor("out", [P, K2, BL], FP32, kind="ExternalOutput").ap()

    with tile.TileContext(nc) as tc:
        with (
            tc.tile_pool(name="const", bufs=1) as cpool,
            tc.tile_pool(name="xt", bufs=4) as xtpool,
            tc.tile_pool(name="h1", bufs=2) as h1pool,
            tc.tile_pool(name="h2", bufs=8) as h2pool,
            tc.tile_pool(name="ps", bufs=4, space="PSUM") as pspool,
        ):
            # --- PE warm-up ---
            # The PE clock ramps to 2.4GHz only after ~3us of sustained
            # activity. Burn the startup-DMA window on cheap dummy matmuls
            # so the real matmuls run fast from the first one.
            warm_sb = cpool.tile([P, N], F16)
            nc.vector.memset(warm_sb[:], 0.0)
            for i in range(12):
                wps = pspool.tile([P, N], FP32, tag="ps", name=f"warm{i}")
                nc.tensor.matmul(wps[:], warm_sb[:, :P], warm_sb[:], start=True, stop=True)

            # --- startup DMAs (issue order = time order on SP) ---
            w1_sb = cpool.tile([P, KK1, 2, D_H], FP8)
            xt_sb = [None] * BL
            for b in range(4):
                xt_sb[b] = xtpool.tile([P, KK1, 2, N], FP8, tag="xt", name=f"xt{b}")
            for kk in range(KK1):
                nc.sync.dma_start(xt_sb[0][:, kk], xt_d[0, :, kk])
                nc.sync.dma_start(w1_sb[:, kk], w1_d[:, kk])
            nc.sync.dma_start(xt_sb[1][:], xt_d[1])
            b1_sb = cpool.tile([P, K2], FP32)
            nc.sync.dma_start(b1_sb[:], b1_d)
            b2_sb = cpool.tile([P, K2], FP32)
            nc.sync.dma_start(b2_sb[:], b2_d)
            w2_sb = cpool.tile([P, KK2, 2, D_H], FP8)
            nc.sync.dma_start(w2_sb[:, : KK2 // 2], w2_d[:, : KK2 // 2])
            nc.sync.dma_start(xt_sb[2][:], xt_d[2])
            nc.sync.dma_start(w2_sb[:, KK2 // 2 :], w2_d[:, KK2 // 2 :])
            nc.sync.dma_start(xt_sb[3][:], xt_d[3])

            # --- persistent SBUF state ---
            pooled = cpool.tile([P, K2, BL], FP32)  # sum-pool accumulators
            pooled_f16 = cpool.tile([P, KR1, BL], F16)  # rho1 rhs: [0:K2]=sum, [K2:]=max
            r1_sb = cpool.tile([P, K2, BL], F16)
            out_sb = cpool.tile([P, K2, BL], FP32)
            wr1_sb = cpool.tile([P, KR1, D_H], F16)
            wr2_sb = cpool.tile([P, K2, D_H], F16)
            br1_sb = cpool.tile([P, K2], FP32)
            br2_sb = cpool.tile([P, K2], FP32)

            def evac(eng, out_ap, ps_ap, bias_ap):
                # relu(psum + bias) -> out, on the named engine
                if eng == "SE":
                    nc.scalar.activation(out_ap, ps_ap, RELU, bias=bias_ap, scale=1.0)
                elif eng == "DVE":
                    nc.vector.tensor_scalar(out_ap, ps_ap, bias_ap, 0.0, ADD, MAX)
                else:
                    nc.gpsimd.tensor_scalar(out_ap, ps_ap, bias_ap, 0.0, ADD, MAX)

            # phi2 tiles on the "accum" route compute their sum-pool via the
            # ScalarE activation accumulator (two per-sample activations)
            # instead of DVE add-reduces, shifting ~2.7us/pair off DVE.
            ACCUM_TILES = (3, 7)

            def phi1_tile(p, m, h1_sb):
                b0 = 2 * p
                ps = pspool.tile([P, 2, N], FP32, tag="ps", name=f"ps1_{p}_{m}")
                for s in (0, 1):
                    for kk in range(KK1):
                        nc.tensor.matmul(
                            ps[:, s],
                            w1_sb[:, kk, :, m * P : (m + 1) * P],
                            xt_sb[b0 + s][:, kk],
                            perf_mode=DR,
                            start=(kk == 0),
                            stop=(kk == KK1 - 1),
                        )
                # both samples share bias b1[m] -> one paired evacuation
                evac("SE", h1_sb[:, m // 2, m % 2], ps[:], b1_sb[:, m : m + 1])

            def phi2_tile(p, m, h1_sb):
                b0 = 2 * p
                ps = pspool.tile([P, 2, N], FP32, tag="ps", name=f"ps2_{p}_{m}")
                for s in (0, 1):
                    for kk in range(KK2):
                        nc.tensor.matmul(
                            ps[:, s],
                            w2_sb[:, kk, :, m * P : (m + 1) * P],
                            h1_sb[:, kk, :, s, :],
                            perf_mode=DR,
                            start=(kk == 0),
                            stop=(kk == KK2 - 1),
                        )
                h2 = h2pool.tile([P, 2, N], F16, tag="h2", name=f"h2_{p}_{m}")
                if m in ACCUM_TILES:
                    # per-sample relu+bias with the ScalarE accumulator giving
                    # the sum-pool directly
                    for s in (0, 1):
                        nc.scalar.activation(
                            h2[:, s],
                            ps[:, s],
                            RELU,
                            bias=b2_sb[:, m : m + 1],
                            scale=1.0,
                            accum_out=pooled[:, m, b0 + s : b0 + s + 1],
                        )
                else:
                    evac("SE", h2[:], ps[:], b2_sb[:, m : m + 1])
                    for s in (0, 1):
                        nc.vector.tensor_reduce(
                            pooled[:, m, b0 + s : b0 + s + 1],
                            h2[:, s],
                            axis=AX_X,
                            op=ADD,
                        )
                nc.vector.tensor_reduce(
                    pooled_f16[:, K2 + m, b0 : b0 + 2], h2[:], axis=AX_X, op=MAX
                )

            # software pipeline, interleaved at tile level: phi1(p)[m] and
            # phi2(p-1)[m] alternate so PSUM slot reuse is spaced ~2.5us of PE
            # work apart and the SE/DVE epilogues arrive evenly. The last
            # pair's max-reduces are deferred past the sum-half cast so the
            # rho1 sum-half matmuls overlap the max drain.
            prev = None
            for p in range(NPAIR):
                for s in (0, 1):
                    b = 2 * p + s
                    if xt_sb[b] is None:
                        xt_sb[b] = xtpool.tile(
                            [P, KK1, 2, N], FP8, tag="xt", name=f"xt{b}"
                        )
                        nc.sync.dma_start(xt_sb[b][:], xt_d[b])
                h1_sb = h1pool.tile([P, KK2, 2, 2, N], FP8, tag="h1", name=f"h1_{p}")
                for m in range(K2):
                    phi1_tile(p, m, h1_sb)
                    if prev is not None:
                        phi2_tile(p - 1, m, prev)
                prev = h1_sb
                if p == 0:
                    nc.sync.dma_start(wr1_sb[:, : KR1 // 4], wr1_d[:, : KR1 // 4])
                elif p == 1:
                    nc.sync.dma_start(
                        wr1_sb[:, KR1 // 4 : KR1 // 2], wr1_d[:, KR1 // 4 : KR1 // 2]
                    )
                elif p == 2:
                    nc.sync.dma_start(wr1_sb[:, KR1 // 2 :], wr1_d[:, KR1 // 2 :])
                    nc.sync.dma_start(wr2_sb[:], wr2_d)
                    nc.sync.dma_start(br1_sb[:], br1_d)
                    nc.sync.dma_start(br2_sb[:], br2_d)
            for m in range(K2):
                phi2_tile(NPAIR - 1, m, prev)

            # sum-pool half -> fp16 for the rho1 matmuls. GPSIMD (SBUF->SBUF
            # is legal there) runs it as soon as the sums land, in parallel
            # with DVE's trailing max-reduces.
            nc.gpsimd.tensor_copy(pooled_f16[:, :K2, :], pooled[:])

            # --- rho MLP over the 8 pooled vectors (feature-major, free=8) ---
            # rho1 in two phases: the sum-half contraction (k<K2) only needs
            # the cast, so it runs on the PE while DVE drains the deferred
            # max-reduces; the max-half joins in phase B. 8 accumulators live
            # in 4 paired PSUM tiles.
            # rho1 in two phases: the sum-half contraction (k<K2) only needs
            # the cast, so the PE runs it while DVE drains the last pair's
            # max-reduces. 8 accumulators live in 4 pair tiles shaped
            # [P, 2, N] so the two chains sit in SEPARATE PSUM banks
            # (accumulate-start within one bank corrupts its neighbor chain).
            ps_r1 = []
            for mq in range(K2 // 2):
                ps = pspool.tile([P, 2, N], FP32, tag="ps", name=f"psr1_{mq}")
                ps_r1.append(ps)
                for j in (0, 1):
                    m = 2 * mq + j
                    for k in range(K2):
                        nc.tensor.matmul(
                            ps[:, j, :BL],
                            wr1_sb[:, k, m * P : (m + 1) * P],
                            pooled_f16[:, k, :],
                            start=(k == 0),
                            stop=False,
                        )
            for mq in range(K2 // 2):
                ps = ps_r1[mq]
                for j in (0, 1):
                    m = 2 * mq + j
                    for k in range(K2, KR1):
                        nc.tensor.matmul(
                            ps[:, j, :BL],
                            wr1_sb[:, k, m * P : (m + 1) * P],
                            pooled_f16[:, k, :],
                            start=False,
                            stop=(k == KR1 - 1),
                        )
                for j in (0, 1):
                    m = 2 * mq + j
                    evac("SE" if m % 2 == 0 else "DVE", r1_sb[:, m, :], ps[:, j, :BL], br1_sb[:, m : m + 1])
            for m in range(K2):
                ps = pspool.tile([P, BL], FP32, tag="ps", name=f"psr2_{m}")
                for k in range(K2):
                    nc.tensor.matmul(
                        ps[:],
                        wr2_sb[:, k, m * P : (m + 1) * P],
                        r1_sb[:, k, :],
                        start=(k == 0),
                        stop=(k == K2 - 1),
                    )
                evac("SE" if m % 2 == 0 else "DVE", out_sb[:, m, :], ps[:], br2_sb[:, m : m + 1])
                if m % 2 == 1:
                    # output leaves in quarters while rho2 finishes
                    nc.sync.dma_start(out_d[:, m - 1 : m + 1], out_sb[:, m - 1 : m + 1])

    return nc


_CACHE: dict = {}


def get_compiled() -> bacc.Bacc:
    if "nc" not in _CACHE:
        nc = build_program()
        nc.compile()
        _CACHE["nc"] = nc
    return _CACHE["nc"]


def stage_inputs(x, W_phi1, b_phi1, W_phi2, b_phi2, W_rho1, b_rho1, W_rho2, b_rho2):
    """Host-side staging: transpose x, cast weights, fp8 bias correction."""

    def wtile16(a):
        # [KO*P, H] -> [P, KO, H] fp16
        a = np.asarray(a, np.float32).astype(NP_F16)
        ko = a.shape[0] // P
        return np.ascontiguousarray(a.reshape(ko, P, -1).transpose(1, 0, 2))

    def w8(a, kk):
        # [kk*256, H] -> [P, kk, 2, H] fp8 with w[p,kk,j,h] = W[kk*256+j*128+p, h]
        q = np.asarray(a, np.float32).astype(NP_FP8)
        return np.ascontiguousarray(q.reshape(kk, 2, P, -1).transpose(2, 0, 1, 3))

    def bias(a):
        # [n_tiles*P] -> [P, n_tiles]
        return np.ascontiguousarray(np.asarray(a, np.float32).reshape(-1, P).T)

    x32 = np.asarray(x, np.float32)
    xq = x32.astype(NP_FP8)
    W1_32 = np.asarray(W_phi1, np.float32)
    W2_32 = np.asarray(W_phi2, np.float32)
    b1_32 = np.asarray(b_phi1, np.float32)
    b2_32 = np.asarray(b_phi2, np.float32)
    W1q = W1_32.astype(NP_FP8).astype(np.float32)
    W2q = W2_32.astype(NP_FP8).astype(np.float32)

    # phi2 fp8 bias correction: h1 is post-relu (positive mean), so the
    # quantization error of (h1, W2) has a data-independent first-order
    # component mean(q(h1)) @ q(W2) - mean(h1) @ W2 that sum-pooling would
    # amplify 512x. Estimate the mean from a few samples and fold the
    # correction into phi2's bias.
    ns = 6
    xs = x32[:ns].reshape(-1, D_IN)
    xsq = xq[:ns].astype(np.float32).reshape(-1, D_IN)
    h1t = np.maximum(xs @ W1_32 + b1_32, 0.0)
    h1q = np.maximum(xsq @ W1q + b1_32, 0.0).astype(NP_FP8).astype(np.float32)
    c = h1q.mean(0) @ W2q - h1t.mean(0) @ W2_32
    b2_eff = b2_32 - c

    # x[b, n, d] -> xt[b, p, kk, j, n] = x[b, n, kk*256 + j*128 + p]  (fp8)
    xt = np.ascontiguousarray(xq.reshape(B, N, KK1, 2, P).transpose(0, 4, 2, 3, 1))
    shared = {
        "w1": w8(W1_32, KK1),
        "w2": w8(W2_32, KK2),
        "wr1": wtile16(W_rho1),
        "wr2": wtile16(W_rho2),
        "b1": bias(b1_32),
        "b2": bias(b2_eff),
        "br1": bias(b_rho1),
        "br2": bias(b_rho2),
    }
    in_maps = []
    for cix in range(N_CORES):
        m = dict(shared)
        m["xt"] = np.ascontiguousarray(xt[cix * BL : (cix + 1) * BL])
        in_maps.append(m)
    return in_maps


def gather_output(results) -> np.ndarray:
    # per-core out: [P, K2, BL] with out[p, m, s] = r2[m*128+p, s]
    parts = []
    for c in range(N_CORES):
        o = np.asarray(results[c]["out"], np.float32)  # [P, K2, BL]
        parts.append(o.transpose(2, 1, 0).reshape(BL, D_H))  # [BL, D_H]
    return np.concatenate(parts, axis=0)


def run(trace: bool = False, **inputs):
    nc = get_compiled()
    in_maps = stage_inputs(**inputs)
    res = run_bass_kernel_spmd(nc, in_maps, core_ids=list(range(N_CORES)), trace=trace)
    return gather_output(res.results), res


def kernel(**inputs) -> np.ndarray:
    out, _ = run(trace=False, **inputs)
    return out
